# revision 12
# baseline (speedup 1.0000x reference)
"""Trainium2 Bass kernel for nn_DiffTime (embedding_lookup, 8 NeuronCores).

Computation (see reference):
    h1 = tanh(times * h1_k + h1_b)            [B, 100]
    tv = tanh(h1 @ h2_k + h2_b)               [B, 100]
    mat_x = (emb_x @ evoke_k + evoke_b)       [B, 100p, 100h]   (x in {target, context})
    mv_x = einsum('bph,bh->bp', mat_x, tv)    [B, 100]
    vect_x = mv_x @ last_k + last_b           [B, 300]
    logits = sum(vect_t * vect_c, -1)         [B]
    out = mean(softplus(logits) - logits * labels)

Strategy: data-parallel over batch (2048 items/core).  Embedding rows are
gathered on-device with dma_gather (int16 indices; the 100k vocab is split
into 4 sorted segments of <=32768 rows, gathered to a DRAM scratch, then a
second gather restores batch order and yields the [e, b] transposed layout
that feeds the tensor engine directly).  The big matmul runs in bf16
(validated: final-scalar error ~2e-6 rel).  The h-contraction runs as
ACT-evict + DVE multiply/reduce.  logits use the Gram matrix
Gh = last_kh @ last_kh.T computed on device (homogeneous coordinate folds
last_b).  Each core returns a partial loss sum; the host adds 8 scalars.
"""

import os
import sys

for _p in ("/opt/trn_rl_repo", "/opt/trn_rl_repo/concourse"):
    if _p not in sys.path:
        sys.path.insert(0, _p)

from contextlib import ExitStack

import ml_dtypes
import numpy as np

import concourse.bacc as bacc
import concourse.bass as bass
import concourse.tile as tile
from concourse import mybir
from concourse.bass_utils import run_bass_kernel_spmd

F32 = mybir.dt.float32
F32R = mybir.dt.float32r
BF16 = mybir.dt.bfloat16
I16 = mybir.dt.int16
AF = mybir.ActivationFunctionType
AX = mybir.AxisListType
OP = mybir.AluOpType

N_CORES = 8
B = 16384
BC = B // N_CORES          # 2048 batch items per core
NB = BC // 128             # 16 chunks of 128 batch rows
V = 100000
EMB = 300
EPAD = 384                 # padded embedding row (col 300 = 1.0, rest 0)
H = 100                    # h1 = h2 = prod dims
NP = 10000                 # prod * h2
CH = 500                   # matmul moving chunk (5 p-groups)
NCH = NP // CH             # 20 chunks
PG = CH // H               # 5 p's per chunk
GRP = 4                    # psum banks used per accumulation group
MH = H + 1                 # homogeneous mv size
SEG_BASE = [0, 32768, 65536, 98304]
SEG_CAP = [768, 768, 768, 128]   # fixed (SPMD-stable) per-segment capacity
S_TOT = sum(SEG_CAP)             # 2432 scratch rows
assert S_TOT % 128 == 0


def _wrap16(v):
    """int16 index array -> dma_gather SBUF layout [128, len//16]."""
    v = np.asarray(v, dtype=np.int16)
    a = v.reshape(-1, 16).T          # [16, len/16]; slot j at [j%16, j//16]
    return np.tile(a, (8, 1))        # replicate across the 8 q7 cores


def _prep_indices(idx):
    """Sort a core's indices into int16-addressable segments.

    Returns (seg_idx [128, S_TOT//16], realign [128, BC//16]) int16 arrays.
    seg_idx holds per-segment local indices (padded with 0); realign maps
    original batch position j -> scratch row of its gathered embedding.
    """
    idx = np.asarray(idx).astype(np.int64)
    assert idx.shape == (BC,)
    order = np.argsort(idx, kind="stable")
    sidx = idx[order]
    bounds = np.searchsorted(sidx, SEG_BASE + [V])
    seg_cols = []
    scratch_rows = np.empty(BC, dtype=np.int64)
    off = 0
    for s in range(4):
        lo, hi = bounds[s], bounds[s + 1]
        n = hi - lo
        assert n <= SEG_CAP[s], f"segment {s} overflow: {n} > {SEG_CAP[s]}"
        local = np.zeros(SEG_CAP[s], dtype=np.int16)
        local[:n] = sidx[lo:hi] - SEG_BASE[s]
        seg_cols.append(_wrap16(local))
        scratch_rows[lo:hi] = off + np.arange(n)
        off += SEG_CAP[s]
    realign = np.empty(BC, dtype=np.int64)
    realign[order] = scratch_rows
    return np.hstack(seg_cols), _wrap16(realign)


def _build_kernel(ctx: ExitStack, tc: "tile.TileContext", io: dict):
    nc = tc.nc

    cpool = ctx.enter_context(tc.tile_pool(name="const", bufs=1))
    gpool = ctx.enter_context(tc.tile_pool(name="gather", bufs=2))
    dpool = ctx.enter_context(tc.tile_pool(name="scratch", bufs=1, space="DRAM"))
    pmm = ctx.enter_context(tc.tile_pool(name="pmm", bufs=GRP, space="PSUM"))
    pmisc = ctx.enter_context(tc.tile_pool(name="pmisc", bufs=3, space="PSUM"))
    wpool = ctx.enter_context(tc.tile_pool(name="work", bufs=2 * GRP))
    tvpool = ctx.enter_context(tc.tile_pool(name="tvp", bufs=3))
    lpool = ctx.enter_context(tc.tile_pool(name="loss", bufs=2))

    # ---- resident constants --------------------------------------------
    evoke = [cpool.tile([128, NP], BF16, tag=f"evoke{j}", name=f"evoke{j}")
             for j in range(3)]
    for j in range(3):
        nc.sync.dma_start(out=evoke[j][:], in_=io["evoke"][128 * j:128 * (j + 1), :])
    h2kb = cpool.tile([H + 1, H], F32, tag="h2kb")
    nc.sync.dma_start(out=h2kb[:], in_=io["h2kb"][:, :])
    h1k = cpool.tile([H, 1], F32, tag="h1k")
    nc.sync.dma_start(out=h1k[:], in_=io["h1k"][:, :])
    h1b = cpool.tile([H, 1], F32, tag="h1b")
    nc.sync.dma_start(out=h1b[:], in_=io["h1b"][:, :])
    lastkh = cpool.tile([MH, EMB], F32, tag="lastkh")
    nc.sync.dma_start(out=lastkh[:], in_=io["lastkh"][:, :])
    ident = cpool.tile([128, 128], F32, tag="ident")
    nc.sync.dma_start(out=ident[:], in_=io["ident"][:, :])
    times = cpool.tile([1, BC], F32, tag="times")
    nc.sync.dma_start(out=times[:], in_=io["times"][:, :])
    labels = cpool.tile([128, NB], F32, tag="labels")
    nc.sync.dma_start(out=labels[:], in_=io["labels"][:, :])
    idx_sb = {}
    for br in ("t", "c"):
        idx_sb[br] = cpool.tile([128, S_TOT // 16], I16, tag=f"idx_{br}", name=f"idx_{br}")
        nc.sync.dma_start(out=idx_sb[br][:], in_=io[f"idx_{br}"][:, :])
        idx_sb[br + "r"] = cpool.tile([128, BC // 16], I16, tag=f"rel_{br}", name=f"rel_{br}")
        nc.sync.dma_start(out=idx_sb[br + "r"][:], in_=io[f"rel_{br}"][:, :])
    ones1 = cpool.tile([1, H], F32, tag="ones1")
    nc.vector.memset(ones1[:], 1.0)
    ones128 = cpool.tile([128, 1], F32, tag="ones128")
    nc.vector.memset(ones128[:], 1.0)

    # ---- gathers: table segments -> scratch -> batch-ordered [e, b] ----
    emb = {}
    for br, tab in (("t", io["ttab"]), ("c", io["ctab"])):
        scratch = dpool.tile([S_TOT, EPAD], BF16, tag=f"scratch_{br}")
        off = 0
        for s in range(4):
            cap = SEG_CAP[s]
            seg_len = min(32768, V - SEG_BASE[s])
            g = gpool.tile([128, 3, cap], BF16, tag="segg")
            nc.gpsimd.dma_gather(
                g[:],
                tab[SEG_BASE[s]:SEG_BASE[s] + seg_len, :],
                idx_sb[br][:, off // 16:(off + cap) // 16],
                cap,
                cap,
                EPAD,
                transpose=True,
                queue_num=0,
            )
            sview = scratch[off:off + cap, :].rearrange(
                "i (j p) -> j p i", p=128, j=3)
            for j in range(3):
                nc.scalar.dma_start(out=sview[j], in_=g[:, j, :])
            off += cap
        e = cpool.tile([128, 3, BC], BF16, tag=f"emb_{br}", name=f"emb_{br}")
        nc.gpsimd.dma_gather(
            e[:], scratch[:, :], idx_sb[br + "r"][:], BC, BC, EPAD,
            transpose=True, queue_num=0, single_packet=False,
        )
        emb[br] = e

    # ---- Gh = last_kh @ last_kh.T on device ----------------------------
    ech = [(0, 128), (128, 128), (256, EMB - 256)]
    lkT = []
    for j, (e0, en) in enumerate(ech):
        tp = pmisc.tile([en, MH], F32, tag="pm")
        nc.tensor.transpose(tp[:], lastkh[:, e0:e0 + en], ident[0:MH, 0:MH])
        t = cpool.tile([en, MH], F32, tag=f"lkT{j}", name=f"lkT{j}")
        nc.scalar.copy(t[:], tp[:])
        lkT.append(t)
    ghp = pmisc.tile([MH, MH], F32, tag="pm")
    for j in range(3):
        a = lkT[j][:]
        nc.tensor.matmul(ghp[:], a, a, start=(j == 0), stop=(j == 2))
    gh = cpool.tile([MH, MH], F32, tag="gh")
    nc.scalar.copy(gh[:], ghp[:])

    # ---- time MLP: tv per batch-chunk (shared by both branches) --------
    tv5_all = []
    for c in range(NB):
        bcast = pmisc.tile([H, 128], F32, tag="pm")
        nc.tensor.matmul(
            bcast[:],
            ones1[:],
            times[0:1, 128 * c:128 * (c + 1)],
            start=True, stop=True,
        )
        h1T = tvpool.tile([H + 1, 128], F32, tag="h1T")
        nc.vector.memset(h1T[:], 1.0)
        nc.scalar.activation(h1T[0:H, :], bcast[:], AF.Tanh, bias=h1b[:], scale=h1k[:])
        tvp = pmisc.tile([128, H], F32, tag="pm")
        nc.tensor.matmul(
            tvp[:], h1T[:], h2kb[:], start=True, stop=True
        )
        tvb = tvpool.tile([128, H], BF16, tag="tvb")
        nc.scalar.activation(tvb[:], tvp[:], AF.Tanh)
        tv5 = cpool.tile([128, CH], BF16, tag=f"tv5_{c}", name=f"tv5_{c}")
        for r in range(PG):
            nc.vector.tensor_copy(tv5[:, H * r:H * (r + 1)], tvb[:])
        tv5_all.append(tv5)

    # ---- main loop: mat = emb.T @ evoke ; mv = sum_h mat * tv ----------
    def branch_mv(br, c):
        mv = cpool.tile([128, MH], F32, tag=f"mv_{br}{c}", name=f"mv_{br}{c}")
        nc.vector.memset(mv[:, H:MH], 1.0)
        lhs = [emb[br][:, j, 128 * c:128 * (c + 1)] for j in range(3)]
        for g0 in range(0, NCH, GRP):
            mps = [pmm.tile([128, CH], F32, tag="mp", name=f"mp{_k}") for _k in range(GRP)]
            for j in range(3):
                for k in range(GRP):
                    n = g0 + k
                    nc.tensor.matmul(
                        mps[k][:], lhs[j], evoke[j][:, CH * n:CH * (n + 1)],
                        start=(j == 0), stop=(j == 2),
                    )
            for k in range(GRP):
                n = g0 + k
                ms = wpool.tile([128, CH], BF16, tag="ms")
                nc.scalar.copy(ms[:], mps[k][:])
                prod = wpool.tile([128, CH], BF16, tag="prod")
                nc.vector.tensor_mul(prod[:], ms[:], tv5_all[c][:])
                nc.vector.reduce_sum(
                    out=mv[:, PG * n:PG * (n + 1)],
                    in_=prod[:].rearrange("p (a h) -> p a h", h=H),
                    axis=AX.X,
                )
        return mv

    stage = os.environ.get("K_STAGE", "full")
    if stage in ("gather", "tv"):
        src_ap = emb["t"][0:1, 0, 0:1] if stage == "gather" else None
        probe = cpool.tile([1, 1], F32, tag="probe")
        if stage == "gather":
            pb = cpool.tile([1, 1], BF16, tag="probeb")
            nc.vector.tensor_copy(pb[:], emb["t"][0:1, 0, 0:1])
            nc.vector.tensor_copy(probe[:], pb[:])
        else:
            nc.vector.tensor_copy(probe[:], tv5_all[NB - 1][0:1, 0:1])
        nc.sync.dma_start(out=io["out"][:, :], in_=probe[:])
        return

    mvt = [branch_mv("t", c) for c in range(NB)]
    if stage == "mvt":
        probe = cpool.tile([1, 1], F32, tag="probe")
        nc.vector.tensor_copy(probe[:], mvt[NB - 1][0:1, 0:1])
        nc.sync.dma_start(out=io["out"][:, :], in_=probe[:])
        return

    dvec = cpool.tile([128, NB], F32, tag="dvec")
    for c in range(NB):
        mvc = branch_mv("c", c)
        tp = pmisc.tile([MH, 128], F32, tag="pm")
        nc.tensor.transpose(tp[:], mvt[c][:], ident[:])
        mvtT = lpool.tile([MH, 128], F32, tag="mvtT")
        nc.scalar.copy(mvtT[:], tp[:])
        mg = pmisc.tile([128, MH], F32, tag="pm")
        nc.tensor.matmul(
            mg[:], mvtT[:], gh[:], start=True, stop=True
        )
        junk = lpool.tile([128, MH], F32, tag="ttrjunk")
        nc.vector.tensor_mul(junk[:], mg[:], mvc[:])
        logit = lpool.tile([128, 1], F32, tag="logit")
        nc.vector.reduce_sum(out=logit[:], in_=junk[:], axis=AX.X)
        # softplus(l) = relu(l) + ln(1 + exp(-|l|))  (no Softplus LUT on trn2)
        ab = lpool.tile([128, 1], F32, tag="ab")
        nc.scalar.activation(ab[:], logit[:], AF.Abs)
        ex = lpool.tile([128, 1], F32, tag="ex")
        nc.scalar.activation(ex[:], ab[:], AF.Exp, scale=-1.0)
        l1p = lpool.tile([128, 1], F32, tag="l1p")
        nc.scalar.activation(l1p[:], ex[:], AF.Ln, bias=1.0)
        rl = lpool.tile([128, 1], F32, tag="rl")
        nc.scalar.activation(rl[:], logit[:], AF.Relu)
        sp = lpool.tile([128, 1], F32, tag="sp")
        nc.vector.tensor_add(sp[:], rl[:], l1p[:])
        ll = lpool.tile([128, 1], F32, tag="ll")
        nc.vector.tensor_mul(ll[:], logit[:], labels[:, c:c + 1])
        nc.vector.tensor_sub(dvec[:, c:c + 1], sp[:], ll[:])

    # ---- reduce 2048 per-sample losses to one partial sum --------------
    srow = cpool.tile([128, 1], F32, tag="srow")
    nc.vector.reduce_sum(out=srow[:], in_=dvec[:], axis=AX.X)
    fin = pmisc.tile([1, 1], F32, tag="pm")
    nc.tensor.matmul(
        fin[:], srow[:], ones128[:], start=True, stop=True
    )
    res = cpool.tile([1, 1], F32, tag="res")
    nc.scalar.copy(res[:], fin[:])
    nc.sync.dma_start(out=io["out"][:, :], in_=res[:])


_PROGRAM = None


def _get_program():
    global _PROGRAM
    if _PROGRAM is not None:
        return _PROGRAM
    nc = bacc.Bacc("TRN2", target_bir_lowering=False, debug=False,
                   num_devices=N_CORES)
    io = {
        "ttab": nc.dram_tensor("ttab", [V, EPAD], BF16, kind="ExternalInput").ap(),
        "ctab": nc.dram_tensor("ctab", [V, EPAD], BF16, kind="ExternalInput").ap(),
        "evoke": nc.dram_tensor("evoke", [EPAD, NP], BF16, kind="ExternalInput").ap(),
        "h2kb": nc.dram_tensor("h2kb", [H + 1, H], F32, kind="ExternalInput").ap(),
        "h1k": nc.dram_tensor("h1k", [H, 1], F32, kind="ExternalInput").ap(),
        "h1b": nc.dram_tensor("h1b", [H, 1], F32, kind="ExternalInput").ap(),
        "lastkh": nc.dram_tensor("lastkh", [MH, EMB], F32, kind="ExternalInput").ap(),
        "ident": nc.dram_tensor("ident", [128, 128], F32, kind="ExternalInput").ap(),
        "times": nc.dram_tensor("times", [1, BC], F32, kind="ExternalInput").ap(),
        "labels": nc.dram_tensor("labels", [128, NB], F32, kind="ExternalInput").ap(),
        "idx_t": nc.dram_tensor("idx_t", [128, S_TOT // 16], I16, kind="ExternalInput").ap(),
        "idx_c": nc.dram_tensor("idx_c", [128, S_TOT // 16], I16, kind="ExternalInput").ap(),
        "rel_t": nc.dram_tensor("rel_t", [128, BC // 16], I16, kind="ExternalInput").ap(),
        "rel_c": nc.dram_tensor("rel_c", [128, BC // 16], I16, kind="ExternalInput").ap(),
        "out": nc.dram_tensor("out", [1, 1], F32, kind="ExternalOutput").ap(),
    }
    with tile.TileContext(nc) as tc:
        with ExitStack() as ctx:
            _build_kernel(ctx, tc, io)
    nc.compile()
    _PROGRAM = nc
    return nc


def _pad_table(tab):
    out = np.zeros((V, EPAD), dtype=ml_dtypes.bfloat16)
    out[:, :EMB] = np.asarray(tab).astype(ml_dtypes.bfloat16)
    out[:, EMB] = 1.0
    return out


def build_in_maps(targets, contexts, times, labels, targetemb, contextemb,
                  h1_k, h1_b, h2_k, h2_b, evoke_k, evoke_b, last_k, last_b):
    ttab = _pad_table(targetemb)
    ctab = _pad_table(contextemb)
    evoke = np.zeros((EPAD, NP), dtype=ml_dtypes.bfloat16)
    evoke[:EMB, :] = np.asarray(evoke_k).astype(ml_dtypes.bfloat16)
    evoke[EMB, :] = np.asarray(evoke_b).astype(ml_dtypes.bfloat16)
    h2kb = np.vstack([np.asarray(h2_k), np.asarray(h2_b).reshape(1, H)]
                     ).astype(np.float32)
    h1kc = np.asarray(h1_k).reshape(1, H).T.astype(np.float32).copy()
    h1bc = np.asarray(h1_b).reshape(H, 1).astype(np.float32).copy()
    lastkh = np.vstack([np.asarray(last_k), np.asarray(last_b).reshape(1, EMB)]
                       ).astype(np.float32)
    ident = np.eye(128, dtype=np.float32)
    targets = np.asarray(targets)
    contexts = np.asarray(contexts)
    times = np.asarray(times).astype(np.float32)
    labels = np.asarray(labels).astype(np.float32)

    in_maps = []
    for k in range(N_CORES):
        sl = slice(k * BC, (k + 1) * BC)
        idx_t, rel_t = _prep_indices(targets[sl])
        idx_c, rel_c = _prep_indices(contexts[sl])
        in_maps.append({
            "ttab": ttab, "ctab": ctab, "evoke": evoke, "h2kb": h2kb,
            "h1k": h1kc, "h1b": h1bc, "lastkh": lastkh, "ident": ident,
            "times": times[sl].reshape(1, BC),
            "labels": labels[sl].reshape(NB, 128).T.copy(),
            "idx_t": idx_t, "idx_c": idx_c, "rel_t": rel_t, "rel_c": rel_c,
        })
    return in_maps


def kernel(**inputs) -> np.ndarray:
    nc = _get_program()
    in_maps = build_in_maps(**inputs)
    r = run_bass_kernel_spmd(nc, in_maps, list(range(N_CORES)))
    total = np.float64(0.0)
    for m in r.results:
        total += np.float64(m["out"][0, 0])
    return np.float32(total / B)


# revision 13
# speedup vs baseline: 8.7037x; 8.7037x over previous
"""Trainium2 Bass kernel for nn_DiffTime (embedding_lookup, 8 NeuronCores).

Computation (see reference):
    h1 = tanh(times * h1_k + h1_b)            [B, 100]
    tv = tanh(h1 @ h2_k + h2_b)               [B, 100]
    mat_x = (emb_x @ evoke_k + evoke_b)       [B, 100p, 100h]   (x in {target, context})
    mv_x = einsum('bph,bh->bp', mat_x, tv)    [B, 100]
    vect_x = mv_x @ last_k + last_b           [B, 300]
    logits = sum(vect_t * vect_c, -1)         [B]
    out = mean(softplus(logits) - logits * labels)

Strategy: data-parallel over batch (2048 items/core).  Embedding rows are
gathered on-device with dma_gather (int16 indices; the 100k vocab is split
into 4 sorted segments of <=32768 rows, gathered to a DRAM scratch, then a
second gather restores batch order and yields the [e, b] transposed layout
that feeds the tensor engine directly).  The big matmul runs in bf16
(validated: final-scalar error ~2e-6 rel).  The h-contraction runs as
ACT-evict + DVE multiply/reduce.  logits use the Gram matrix
Gh = last_kh @ last_kh.T computed on device (homogeneous coordinate folds
last_b).  Each core returns a partial loss sum; the host adds 8 scalars.
"""

import os
import sys

for _p in ("/opt/trn_rl_repo", "/opt/trn_rl_repo/concourse"):
    if _p not in sys.path:
        sys.path.insert(0, _p)

from contextlib import ExitStack

import ml_dtypes
import numpy as np

import concourse.bacc as bacc
import concourse.bass as bass
import concourse.tile as tile
from concourse import mybir
from concourse.bass_utils import run_bass_kernel_spmd

F32 = mybir.dt.float32
F32R = mybir.dt.float32r
BF16 = mybir.dt.bfloat16
I16 = mybir.dt.int16
AF = mybir.ActivationFunctionType
AX = mybir.AxisListType
OP = mybir.AluOpType

N_CORES = 8
B = 16384
BC = B // N_CORES          # 2048 batch items per core
NB = BC // 128             # 16 chunks of 128 batch rows
V = 100000
EMB = 300
EPAD = 384                 # padded embedding row (col 300 = 1.0, rest 0)
H = 100                    # h1 = h2 = prod dims
NP = 10000                 # prod * h2
CH = 500                   # matmul moving chunk (5 p-groups)
NCH = NP // CH             # 20 chunks
PG = CH // H               # 5 p's per chunk
GRP = 4                    # psum banks used per accumulation group
MH = H + 1                 # homogeneous mv size
SEG_BASE = [0, 32768, 65536, 98304]
SEG_CAP = [768, 768, 768, 128]   # fixed (SPMD-stable) per-segment capacity
S_TOT = sum(SEG_CAP)             # 2432 scratch rows
assert S_TOT % 128 == 0


def _wrap16(v):
    """int16 index array -> dma_gather SBUF layout [128, len//16]."""
    v = np.asarray(v, dtype=np.int16)
    a = v.reshape(-1, 16).T          # [16, len/16]; slot j at [j%16, j//16]
    return np.tile(a, (8, 1))        # replicate across the 8 q7 cores


def _prep_indices(idx):
    """Sort a core's indices into int16-addressable segments.

    Returns (seg_idx [128, S_TOT//16], realign [128, BC//16]) int16 arrays.
    seg_idx holds per-segment local indices (padded with 0); realign maps
    original batch position j -> scratch row of its gathered embedding.
    """
    idx = np.asarray(idx).astype(np.int64)
    assert idx.shape == (BC,)
    order = np.argsort(idx, kind="stable")
    sidx = idx[order]
    bounds = np.searchsorted(sidx, SEG_BASE + [V])
    seg_cols = []
    scratch_rows = np.empty(BC, dtype=np.int64)
    off = 0
    for s in range(4):
        lo, hi = bounds[s], bounds[s + 1]
        n = hi - lo
        assert n <= SEG_CAP[s], f"segment {s} overflow: {n} > {SEG_CAP[s]}"
        local = np.zeros(SEG_CAP[s], dtype=np.int16)
        local[:n] = sidx[lo:hi] - SEG_BASE[s]
        seg_cols.append(_wrap16(local))
        scratch_rows[lo:hi] = off + np.arange(n)
        off += SEG_CAP[s]
    realign = np.empty(BC, dtype=np.int64)
    realign[order] = scratch_rows
    return np.hstack(seg_cols), _wrap16(realign)


def _build_kernel(ctx: ExitStack, tc: "tile.TileContext", io: dict):
    nc = tc.nc

    cpool = ctx.enter_context(tc.tile_pool(name="const", bufs=1))
    gpool = ctx.enter_context(tc.tile_pool(name="gather", bufs=2))
    dpool = ctx.enter_context(tc.tile_pool(name="scratch", bufs=1, space="DRAM"))
    pmm = ctx.enter_context(tc.tile_pool(name="pmm", bufs=GRP, space="PSUM"))
    pmisc = ctx.enter_context(tc.tile_pool(name="pmisc", bufs=3, space="PSUM"))
    wpool = ctx.enter_context(tc.tile_pool(name="work", bufs=2 * GRP))
    tvpool = ctx.enter_context(tc.tile_pool(name="tvp", bufs=3))
    lpool = ctx.enter_context(tc.tile_pool(name="loss", bufs=2))

    # ---- resident constants --------------------------------------------
    evoke = [cpool.tile([128, NP], BF16, tag=f"evoke{j}", name=f"evoke{j}")
             for j in range(3)]
    for j in range(3):
        nc.sync.dma_start(out=evoke[j][:], in_=io["evoke"][128 * j:128 * (j + 1), :])
    h2kb = cpool.tile([H + 1, H], F32, tag="h2kb")
    nc.sync.dma_start(out=h2kb[:], in_=io["h2kb"][:, :])
    h1k = cpool.tile([H, 1], F32, tag="h1k")
    nc.sync.dma_start(out=h1k[:], in_=io["h1k"][:, :])
    h1b = cpool.tile([H, 1], F32, tag="h1b")
    nc.sync.dma_start(out=h1b[:], in_=io["h1b"][:, :])
    lastkh = cpool.tile([MH, EMB], F32, tag="lastkh")
    nc.sync.dma_start(out=lastkh[:], in_=io["lastkh"][:, :])
    ident = cpool.tile([128, 128], F32, tag="ident")
    nc.sync.dma_start(out=ident[:], in_=io["ident"][:, :])
    identb = cpool.tile([128, 128], BF16, tag="identb")
    nc.sync.dma_start(out=identb[:], in_=io["identb"][:, :])
    times = cpool.tile([1, BC], F32, tag="times")
    nc.sync.dma_start(out=times[:], in_=io["times"][:, :])
    labels = cpool.tile([128, NB], F32, tag="labels")
    nc.sync.dma_start(out=labels[:], in_=io["labels"][:, :])
    idx_sb = {}
    for br in ("t", "c"):
        idx_sb[br] = cpool.tile([128, S_TOT // 16], I16, tag=f"idx_{br}", name=f"idx_{br}")
        nc.sync.dma_start(out=idx_sb[br][:], in_=io[f"idx_{br}"][:, :])
        idx_sb[br + "r"] = cpool.tile([128, BC // 16], I16, tag=f"rel_{br}", name=f"rel_{br}")
        nc.sync.dma_start(out=idx_sb[br + "r"][:], in_=io[f"rel_{br}"][:, :])
    ones1 = cpool.tile([1, H], F32, tag="ones1")
    nc.vector.memset(ones1[:], 1.0)
    ones128 = cpool.tile([128, 1], F32, tag="ones128")
    nc.vector.memset(ones128[:], 1.0)

    # ---- gathers: table segments -> scratch -> batch-ordered rows -------
    # Non-transpose mode: one contiguous 768B descriptor per row (transpose
    # mode shatters each row into 2-byte-granularity writes -- a 1.9M
    # descriptor storm that made DMA the bottleneck).  Row-major [128, c,
    # 384] tiles; the [e, b] lhsT layout is recovered per chunk with PE
    # transposes.
    emb = {}
    for br, tab in (("t", io["ttab"]), ("c", io["ctab"])):
        scratch = dpool.tile([S_TOT, EPAD], BF16, tag=f"scratch_{br}")
        off = 0
        for s in range(4):
            cap = SEG_CAP[s]
            seg_len = min(32768, V - SEG_BASE[s])
            g = gpool.tile([128, cap // 128, EPAD], BF16, tag="segg")
            nc.gpsimd.dma_gather(
                g[:],
                tab[SEG_BASE[s]:SEG_BASE[s] + seg_len, :],
                idx_sb[br][:, off // 16:(off + cap) // 16],
                cap,
                cap,
                EPAD,
                queue_num=0,
            )
            sview = scratch[off:off + cap, :].rearrange(
                "(c p) e -> p c e", p=128)
            nc.scalar.dma_start(out=sview, in_=g[:])
            off += cap
        e = cpool.tile([128, NB, EPAD], BF16, tag=f"emb_{br}", name=f"emb_{br}")
        nc.gpsimd.dma_gather(
            e[:], scratch[:, :], idx_sb[br + "r"][:], BC, BC, EPAD,
            queue_num=0, single_packet=False,
        )
        emb[br] = e

    # ---- Gh = last_kh @ last_kh.T on device ----------------------------
    ech = [(0, 128), (128, 128), (256, EMB - 256)]
    lkT = []
    for j, (e0, en) in enumerate(ech):
        tp = pmisc.tile([en, MH], F32, tag="pm")
        nc.tensor.transpose(tp[:], lastkh[:, e0:e0 + en], ident[0:MH, 0:MH])
        t = cpool.tile([en, MH], F32, tag=f"lkT{j}", name=f"lkT{j}")
        nc.scalar.copy(t[:], tp[:])
        lkT.append(t)
    ghp = pmisc.tile([MH, MH], F32, tag="pm")
    for j in range(3):
        a = lkT[j][:]
        nc.tensor.matmul(ghp[:], a, a, start=(j == 0), stop=(j == 2))
    gh = cpool.tile([MH, MH], F32, tag="gh")
    nc.scalar.copy(gh[:], ghp[:])

    # ---- time MLP: tv per batch-chunk (shared by both branches) --------
    tv5_all = []
    for c in range(NB):
        bcast = pmisc.tile([H, 128], F32, tag="pm")
        nc.tensor.matmul(
            bcast[:],
            ones1[:],
            times[0:1, 128 * c:128 * (c + 1)],
            start=True, stop=True,
        )
        h1T = tvpool.tile([H + 1, 128], F32, tag="h1T")
        nc.vector.memset(h1T[:], 1.0)
        nc.scalar.activation(h1T[0:H, :], bcast[:], AF.Tanh, bias=h1b[:], scale=h1k[:])
        tvp = pmisc.tile([128, H], F32, tag="pm")
        nc.tensor.matmul(
            tvp[:], h1T[:], h2kb[:], start=True, stop=True
        )
        tvb = tvpool.tile([128, H], BF16, tag="tvb")
        nc.scalar.activation(tvb[:], tvp[:], AF.Tanh)
        tv5 = cpool.tile([128, CH], BF16, tag=f"tv5_{c}", name=f"tv5_{c}")
        for r in range(PG):
            nc.vector.tensor_copy(tv5[:, H * r:H * (r + 1)], tvb[:])
        tv5_all.append(tv5)

    # ---- main loop: mat = emb.T @ evoke ; mv = sum_h mat * tv ----------
    def branch_mv(br, c):
        mv = cpool.tile([128, MH], F32, tag=f"mv_{br}{c}", name=f"mv_{br}{c}")
        nc.vector.memset(mv[:, H:MH], 1.0)
        lhs = []
        for j in range(3):
            tpp = pmisc.tile([128, 128], BF16, tag="pm", name=f"ptr{j}")
            nc.tensor.transpose(
                tpp[:], emb[br][:, c, 128 * j:128 * (j + 1)], identb[:])
            et = wpool.tile([128, 128], BF16, tag=f"embT{j}", name=f"embT{j}")
            nc.scalar.copy(et[:], tpp[:])
            lhs.append(et[:])
        for g0 in range(0, NCH, GRP):
            mps = [pmm.tile([128, CH], F32, tag="mp", name=f"mp{_k}") for _k in range(GRP)]
            for j in range(3):
                for k in range(GRP):
                    n = g0 + k
                    nc.tensor.matmul(
                        mps[k][:], lhs[j], evoke[j][:, CH * n:CH * (n + 1)],
                        start=(j == 0), stop=(j == 2),
                    )
            for k in range(GRP):
                n = g0 + k
                ms = wpool.tile([128, CH], BF16, tag="ms")
                nc.scalar.copy(ms[:], mps[k][:])
                prod = wpool.tile([128, CH], BF16, tag="prod")
                nc.vector.tensor_mul(prod[:], ms[:], tv5_all[c][:])
                nc.vector.reduce_sum(
                    out=mv[:, PG * n:PG * (n + 1)],
                    in_=prod[:].rearrange("p (a h) -> p a h", h=H),
                    axis=AX.X,
                )
        return mv

    stage = os.environ.get("K_STAGE", "full")
    if stage in ("gather", "tv"):
        src_ap = emb["t"][0:1, 0, 0:1] if stage == "gather" else None
        probe = cpool.tile([1, 1], F32, tag="probe")
        if stage == "gather":
            pb = cpool.tile([1, 1], BF16, tag="probeb")
            nc.vector.tensor_copy(pb[:], emb["t"][0:1, 0, 0:1])
            nc.vector.tensor_copy(probe[:], pb[:])
        else:
            nc.vector.tensor_copy(probe[:], tv5_all[NB - 1][0:1, 0:1])
        nc.sync.dma_start(out=io["out"][:, :], in_=probe[:])
        return

    mvt = [branch_mv("t", c) for c in range(NB)]
    if stage == "mvt":
        probe = cpool.tile([1, 1], F32, tag="probe")
        nc.vector.tensor_copy(probe[:], mvt[NB - 1][0:1, 0:1])
        nc.sync.dma_start(out=io["out"][:, :], in_=probe[:])
        return

    dvec = cpool.tile([128, NB], F32, tag="dvec")
    for c in range(NB):
        mvc = branch_mv("c", c)
        tp = pmisc.tile([MH, 128], F32, tag="pm")
        nc.tensor.transpose(tp[:], mvt[c][:], ident[:])
        mvtT = lpool.tile([MH, 128], F32, tag="mvtT")
        nc.scalar.copy(mvtT[:], tp[:])
        mg = pmisc.tile([128, MH], F32, tag="pm")
        nc.tensor.matmul(
            mg[:], mvtT[:], gh[:], start=True, stop=True
        )
        junk = lpool.tile([128, MH], F32, tag="ttrjunk")
        nc.vector.tensor_mul(junk[:], mg[:], mvc[:])
        logit = lpool.tile([128, 1], F32, tag="logit")
        nc.vector.reduce_sum(out=logit[:], in_=junk[:], axis=AX.X)
        # softplus(l) = relu(l) + ln(1 + exp(-|l|))  (no Softplus LUT on trn2)
        ab = lpool.tile([128, 1], F32, tag="ab")
        nc.scalar.activation(ab[:], logit[:], AF.Abs)
        ex = lpool.tile([128, 1], F32, tag="ex")
        nc.scalar.activation(ex[:], ab[:], AF.Exp, scale=-1.0)
        l1p = lpool.tile([128, 1], F32, tag="l1p")
        nc.scalar.activation(l1p[:], ex[:], AF.Ln, bias=1.0)
        rl = lpool.tile([128, 1], F32, tag="rl")
        nc.scalar.activation(rl[:], logit[:], AF.Relu)
        sp = lpool.tile([128, 1], F32, tag="sp")
        nc.vector.tensor_add(sp[:], rl[:], l1p[:])
        ll = lpool.tile([128, 1], F32, tag="ll")
        nc.vector.tensor_mul(ll[:], logit[:], labels[:, c:c + 1])
        nc.vector.tensor_sub(dvec[:, c:c + 1], sp[:], ll[:])

    # ---- reduce 2048 per-sample losses to one partial sum --------------
    srow = cpool.tile([128, 1], F32, tag="srow")
    nc.vector.reduce_sum(out=srow[:], in_=dvec[:], axis=AX.X)
    fin = pmisc.tile([1, 1], F32, tag="pm")
    nc.tensor.matmul(
        fin[:], srow[:], ones128[:], start=True, stop=True
    )
    res = cpool.tile([1, 1], F32, tag="res")
    nc.scalar.copy(res[:], fin[:])
    nc.sync.dma_start(out=io["out"][:, :], in_=res[:])


_PROGRAM = None


def _get_program():
    global _PROGRAM
    if _PROGRAM is not None:
        return _PROGRAM
    nc = bacc.Bacc("TRN2", target_bir_lowering=False, debug=False,
                   num_devices=N_CORES)
    io = {
        "ttab": nc.dram_tensor("ttab", [V, EPAD], BF16, kind="ExternalInput").ap(),
        "ctab": nc.dram_tensor("ctab", [V, EPAD], BF16, kind="ExternalInput").ap(),
        "evoke": nc.dram_tensor("evoke", [EPAD, NP], BF16, kind="ExternalInput").ap(),
        "h2kb": nc.dram_tensor("h2kb", [H + 1, H], F32, kind="ExternalInput").ap(),
        "h1k": nc.dram_tensor("h1k", [H, 1], F32, kind="ExternalInput").ap(),
        "h1b": nc.dram_tensor("h1b", [H, 1], F32, kind="ExternalInput").ap(),
        "lastkh": nc.dram_tensor("lastkh", [MH, EMB], F32, kind="ExternalInput").ap(),
        "ident": nc.dram_tensor("ident", [128, 128], F32, kind="ExternalInput").ap(),
        "identb": nc.dram_tensor("identb", [128, 128], BF16, kind="ExternalInput").ap(),
        "times": nc.dram_tensor("times", [1, BC], F32, kind="ExternalInput").ap(),
        "labels": nc.dram_tensor("labels", [128, NB], F32, kind="ExternalInput").ap(),
        "idx_t": nc.dram_tensor("idx_t", [128, S_TOT // 16], I16, kind="ExternalInput").ap(),
        "idx_c": nc.dram_tensor("idx_c", [128, S_TOT // 16], I16, kind="ExternalInput").ap(),
        "rel_t": nc.dram_tensor("rel_t", [128, BC // 16], I16, kind="ExternalInput").ap(),
        "rel_c": nc.dram_tensor("rel_c", [128, BC // 16], I16, kind="ExternalInput").ap(),
        "out": nc.dram_tensor("out", [1, 1], F32, kind="ExternalOutput").ap(),
    }
    with tile.TileContext(nc) as tc:
        with ExitStack() as ctx:
            _build_kernel(ctx, tc, io)
    nc.compile()
    _PROGRAM = nc
    return nc


def _pad_table(tab):
    out = np.zeros((V, EPAD), dtype=ml_dtypes.bfloat16)
    out[:, :EMB] = np.asarray(tab).astype(ml_dtypes.bfloat16)
    out[:, EMB] = 1.0
    return out


def build_in_maps(targets, contexts, times, labels, targetemb, contextemb,
                  h1_k, h1_b, h2_k, h2_b, evoke_k, evoke_b, last_k, last_b):
    ttab = _pad_table(targetemb)
    ctab = _pad_table(contextemb)
    evoke = np.zeros((EPAD, NP), dtype=ml_dtypes.bfloat16)
    evoke[:EMB, :] = np.asarray(evoke_k).astype(ml_dtypes.bfloat16)
    evoke[EMB, :] = np.asarray(evoke_b).astype(ml_dtypes.bfloat16)
    h2kb = np.vstack([np.asarray(h2_k), np.asarray(h2_b).reshape(1, H)]
                     ).astype(np.float32)
    h1kc = np.asarray(h1_k).reshape(1, H).T.astype(np.float32).copy()
    h1bc = np.asarray(h1_b).reshape(H, 1).astype(np.float32).copy()
    lastkh = np.vstack([np.asarray(last_k), np.asarray(last_b).reshape(1, EMB)]
                       ).astype(np.float32)
    ident = np.eye(128, dtype=np.float32)
    identb = np.eye(128, dtype=ml_dtypes.bfloat16)
    targets = np.asarray(targets)
    contexts = np.asarray(contexts)
    times = np.asarray(times).astype(np.float32)
    labels = np.asarray(labels).astype(np.float32)

    in_maps = []
    for k in range(N_CORES):
        sl = slice(k * BC, (k + 1) * BC)
        idx_t, rel_t = _prep_indices(targets[sl])
        idx_c, rel_c = _prep_indices(contexts[sl])
        in_maps.append({
            "ttab": ttab, "ctab": ctab, "evoke": evoke, "h2kb": h2kb,
            "h1k": h1kc, "h1b": h1bc, "lastkh": lastkh, "ident": ident,
            "identb": identb,
            "times": times[sl].reshape(1, BC),
            "labels": labels[sl].reshape(NB, 128).T.copy(),
            "idx_t": idx_t, "idx_c": idx_c, "rel_t": rel_t, "rel_c": rel_c,
        })
    return in_maps


def kernel(**inputs) -> np.ndarray:
    nc = _get_program()
    in_maps = build_in_maps(**inputs)
    r = run_bass_kernel_spmd(nc, in_maps, list(range(N_CORES)))
    total = np.float64(0.0)
    for m in r.results:
        total += np.float64(m["out"][0, 0])
    return np.float32(total / B)


# revision 14
# speedup vs baseline: 21.8986x; 2.5160x over previous
"""Trainium2 Bass kernel for nn_DiffTime (embedding_lookup, 8 NeuronCores).

Computation (see reference):
    h1 = tanh(times * h1_k + h1_b)            [B, 100]
    tv = tanh(h1 @ h2_k + h2_b)               [B, 100]
    mat_x = (emb_x @ evoke_k + evoke_b)       [B, 100p, 100h]   (x in {target, context})
    mv_x = einsum('bph,bh->bp', mat_x, tv)    [B, 100]
    vect_x = mv_x @ last_k + last_b           [B, 300]
    logits = sum(vect_t * vect_c, -1)         [B]
    out = mean(softplus(logits) - logits * labels)

Strategy (data-parallel over batch, 2048 items/core, no collectives):

* Embedding rows are gathered on-device with dma_gather (int16 indices:
  the 100k vocab is split into 4 sorted segments of <=32768 rows, gathered
  to a DRAM scratch, then a second gather restores batch order).  Gathers
  run in row-major (non-transpose) mode -- one contiguous descriptor per
  row; the [e, b] lhsT layout is recovered with PE transposes per chunk.

* tv[b,:] is a function of the single scalar times[b], so its rows live on
  a smooth 1-D curve in R^100.  The curve's SVD (host precompute from the
  MLP weights only -- input independent) collapses: rank 16 reproduces tv
  to ~3e-11.  The kernel therefore contracts emb with
  Wr[e,(p,k)] = sum_h evoke[e,p*100+h]*Vr[h,k]  (k = 16 basis coeffs)
  and forms mv[b,p] = sum_k matU[b,p,k] * c[b,k], c = tv @ Vr -- an
  r=16 contraction instead of 100, cutting TensorE+VectorE work ~6x.
  Validated end-to-end (bf16 pipeline): final-scalar rel err 1.2e-6.

* logits use the Gram matrix Gh = last_kh @ last_kh.T computed on device
  (homogeneous coordinate folds last_b).  Per-sample losses are computed
  batched [128, 16] at the end (softplus via Relu + Ln(1+Exp(-|x|))); each
  core returns a partial sum and the host adds 8 scalars.
"""

import sys

for _p in ("/opt/trn_rl_repo", "/opt/trn_rl_repo/concourse"):
    if _p not in sys.path:
        sys.path.insert(0, _p)

from contextlib import ExitStack

import ml_dtypes
import numpy as np

import concourse.bacc as bacc
import concourse.bass as bass
import concourse.tile as tile
from concourse import mybir
from concourse.bass_utils import run_bass_kernel_spmd

F32 = mybir.dt.float32
BF16 = mybir.dt.bfloat16
I16 = mybir.dt.int16
AF = mybir.ActivationFunctionType
AX = mybir.AxisListType
OP = mybir.AluOpType

N_CORES = 8
B = 16384
BC = B // N_CORES          # 2048 batch items per core
NB = BC // 128             # 16 chunks of 128 batch rows
V = 100000
EMB = 300
EPAD = 384                 # padded embedding row (col 300 = 1.0, rest 0)
H = 100                    # h1 = h2 = prod dims
R = 16                     # tv-curve basis rank
NPR = H * R                # 1600 contracted columns
CH = 25 * R                # 400: matmul moving chunk (25 p-groups)
NCH = NPR // CH            # 4 chunks
PG = CH // R               # 25 p's per chunk
MH = H + 1                 # homogeneous mv size
SEG_BASE = [0, 32768, 65536, 98304]
SEG_CAP = [768, 768, 768, 128]   # fixed (SPMD-stable) per-segment capacity
S_TOT = sum(SEG_CAP)             # 2432 scratch rows
assert S_TOT % 128 == 0


def _wrap16(v):
    """int16 index array -> dma_gather SBUF layout [128, len//16]."""
    v = np.asarray(v, dtype=np.int16)
    a = v.reshape(-1, 16).T          # [16, len/16]; slot j at [j%16, j//16]
    return np.tile(a, (8, 1))        # replicate across the 8 q7 cores


def _prep_indices(idx):
    """Sort a core's indices into int16-addressable segments.

    Returns (seg_idx [128, S_TOT//16], realign [128, BC//16]) int16 arrays.
    seg_idx holds per-segment local indices (padded with 0); realign maps
    original batch position j -> scratch row of its gathered embedding.
    """
    idx = np.asarray(idx).astype(np.int64)
    assert idx.shape == (BC,)
    order = np.argsort(idx, kind="stable")
    sidx = idx[order]
    bounds = np.searchsorted(sidx, SEG_BASE + [V])
    seg_cols = []
    scratch_rows = np.empty(BC, dtype=np.int64)
    off = 0
    for s in range(4):
        lo, hi = bounds[s], bounds[s + 1]
        n = hi - lo
        assert n <= SEG_CAP[s], f"segment {s} overflow: {n} > {SEG_CAP[s]}"
        local = np.zeros(SEG_CAP[s], dtype=np.int16)
        local[:n] = sidx[lo:hi] - SEG_BASE[s]
        seg_cols.append(_wrap16(local))
        scratch_rows[lo:hi] = off + np.arange(n)
        off += SEG_CAP[s]
    realign = np.empty(BC, dtype=np.int64)
    realign[order] = scratch_rows
    return np.hstack(seg_cols), _wrap16(realign)


def _build_kernel(ctx: ExitStack, tc: "tile.TileContext", io: dict):
    nc = tc.nc

    cpool = ctx.enter_context(tc.tile_pool(name="const", bufs=1))
    gpool = ctx.enter_context(tc.tile_pool(name="gather", bufs=2))
    dpool = ctx.enter_context(tc.tile_pool(name="scratch", bufs=1, space="DRAM"))
    pmm = ctx.enter_context(tc.tile_pool(name="pmm", bufs=4, space="PSUM"))
    pmisc = ctx.enter_context(tc.tile_pool(name="pmisc", bufs=3, space="PSUM"))
    wpool = ctx.enter_context(tc.tile_pool(name="work", bufs=4))
    tvpool = ctx.enter_context(tc.tile_pool(name="tvp", bufs=3))
    lpool = ctx.enter_context(tc.tile_pool(name="loss", bufs=2))

    # ---- resident constants --------------------------------------------
    wr = [cpool.tile([128, NPR], BF16, tag=f"wr{j}", name=f"wr{j}")
          for j in range(3)]
    for j in range(3):
        nc.sync.dma_start(out=wr[j][:], in_=io["wr"][128 * j:128 * (j + 1), :])
    h2kb = cpool.tile([H + 1, H], F32, tag="h2kb")
    nc.sync.dma_start(out=h2kb[:], in_=io["h2kb"][:, :])
    h1k = cpool.tile([H, 1], F32, tag="h1k")
    nc.sync.dma_start(out=h1k[:], in_=io["h1k"][:, :])
    h1b = cpool.tile([H, 1], F32, tag="h1b")
    nc.sync.dma_start(out=h1b[:], in_=io["h1b"][:, :])
    vr = cpool.tile([H, R], F32, tag="vr")
    nc.sync.dma_start(out=vr[:], in_=io["vr"][:, :])
    lastkh = cpool.tile([MH, EMB], F32, tag="lastkh")
    nc.sync.dma_start(out=lastkh[:], in_=io["lastkh"][:, :])
    ident = cpool.tile([128, 128], F32, tag="ident")
    nc.sync.dma_start(out=ident[:], in_=io["ident"][:, :])
    identb = cpool.tile([128, 128], BF16, tag="identb")
    nc.sync.dma_start(out=identb[:], in_=io["identb"][:, :])
    times = cpool.tile([1, BC], F32, tag="times")
    nc.sync.dma_start(out=times[:], in_=io["times"][:, :])
    labels = cpool.tile([128, NB], F32, tag="labels")
    nc.sync.dma_start(out=labels[:], in_=io["labels"][:, :])
    idx_sb = {}
    for br in ("t", "c"):
        idx_sb[br] = cpool.tile([128, S_TOT // 16], I16, tag=f"idx_{br}",
                                name=f"idx_{br}")
        nc.sync.dma_start(out=idx_sb[br][:], in_=io[f"idx_{br}"][:, :])
        idx_sb[br + "r"] = cpool.tile([128, BC // 16], I16, tag=f"rel_{br}",
                                      name=f"rel_{br}")
        nc.sync.dma_start(out=idx_sb[br + "r"][:], in_=io[f"rel_{br}"][:, :])
    ones1 = cpool.tile([1, H], F32, tag="ones1")
    nc.vector.memset(ones1[:], 1.0)
    ones128 = cpool.tile([128, 1], F32, tag="ones128")
    nc.vector.memset(ones128[:], 1.0)

    # ---- gathers: table segments -> scratch -> batch-ordered rows ------
    # Row-major mode keeps one contiguous 768B descriptor per row.  Segment
    # gathers for both branches are issued before either realign so the
    # gpsimd queue stays busy while evict DMAs complete.
    scratch = {}
    for br, tab in (("t", io["ttab"]), ("c", io["ctab"])):
        scratch[br] = dpool.tile([S_TOT, EPAD], BF16, tag=f"scratch_{br}",
                                 name=f"scratch_{br}")
        off = 0
        for s in range(4):
            cap = SEG_CAP[s]
            seg_len = min(32768, V - SEG_BASE[s])
            g = gpool.tile([128, cap // 128, EPAD], BF16, tag="segg",
                           name=f"segg_{br}{s}")
            nc.gpsimd.dma_gather(
                g[:],
                tab[SEG_BASE[s]:SEG_BASE[s] + seg_len, :],
                idx_sb[br][:, off // 16:(off + cap) // 16],
                cap, cap, EPAD, queue_num=0,
            )
            sview = scratch[br][off:off + cap, :].rearrange(
                "(c p) e -> p c e", p=128)
            nc.scalar.dma_start(out=sview, in_=g[:])
            off += cap
    emb = {}
    for br in ("t", "c"):
        e = cpool.tile([128, NB, EPAD], BF16, tag=f"emb_{br}", name=f"emb_{br}")
        nc.gpsimd.dma_gather(
            e[:], scratch[br][:, :], idx_sb[br + "r"][:], BC, BC, EPAD,
            queue_num=0, single_packet=False,
        )
        emb[br] = e

    # ---- Gh = last_kh @ last_kh.T on device ----------------------------
    ech = [(0, 128), (128, 128), (256, EMB - 256)]
    lkT = []
    for j, (e0, en) in enumerate(ech):
        tp = pmisc.tile([en, MH], F32, tag="pm", name=f"ptrans{j}")
        nc.tensor.transpose(tp[:], lastkh[:, e0:e0 + en], ident[0:MH, 0:MH])
        t = cpool.tile([en, MH], F32, tag=f"lkT{j}", name=f"lkT{j}")
        nc.scalar.copy(t[:], tp[:])
        lkT.append(t)
    ghp = pmisc.tile([MH, MH], F32, tag="pm")
    for j in range(3):
        nc.tensor.matmul(ghp[:], lkT[j][:], lkT[j][:], start=(j == 0),
                         stop=(j == 2))
    gh = cpool.tile([MH, MH], F32, tag="gh")
    nc.scalar.copy(gh[:], ghp[:])

    # ---- time MLP -> tv basis coefficients c[b, :R] per chunk ----------
    c_all = []
    for c in range(NB):
        bcast = pmisc.tile([H, 128], F32, tag="pm", name="pbcast")
        nc.tensor.matmul(bcast[:], ones1[:],
                         times[0:1, 128 * c:128 * (c + 1)],
                         start=True, stop=True)
        h1T = tvpool.tile([H + 1, 128], F32, tag="h1T")
        nc.vector.memset(h1T[:], 1.0)
        nc.scalar.activation(h1T[0:H, :], bcast[:], AF.Tanh, bias=h1b[:],
                             scale=h1k[:])
        tvp = pmisc.tile([H, 128], F32, tag="pm", name="ptv")
        nc.tensor.matmul(tvp[:], h2kb[:], h1T[:], start=True, stop=True)
        tvT = tvpool.tile([H, 128], F32, tag="tvT")
        nc.scalar.activation(tvT[:], tvp[:], AF.Tanh)
        cfp = pmisc.tile([128, R], F32, tag="pm", name="pcf")
        nc.tensor.matmul(cfp[:], tvT[:], vr[:], start=True, stop=True)
        cb = tvpool.tile([128, R], BF16, tag="cb")
        nc.scalar.copy(cb[:], cfp[:])
        ctile = cpool.tile([128, CH], BF16, tag=f"ct_{c}", name=f"ct_{c}")
        nc.vector.tensor_copy(ctile[:, 0:R], cb[:])
        w = R
        while w < CH:
            n = min(w, CH - w)
            nc.vector.tensor_copy(ctile[:, w:w + n], ctile[:, 0:n])
            w += n
        c_all.append(ctile)

    # ---- main loop: matU = embT @ Wr ; mv = sum_k matU * c -------------
    def branch_mv(br, c):
        mv = cpool.tile([128, MH], F32, tag=f"mv_{br}{c}", name=f"mv_{br}{c}")
        nc.vector.memset(mv[:, H:MH], 1.0)
        lhs = []
        for j in range(3):
            tpp = pmisc.tile([128, 128], BF16, tag="pm", name=f"ptr{j}")
            nc.tensor.transpose(
                tpp[:], emb[br][:, c, 128 * j:128 * (j + 1)], identb[:])
            et = wpool.tile([128, 128], BF16, tag=f"embT{j}", name=f"embT{j}")
            nc.scalar.copy(et[:], tpp[:])
            lhs.append(et[:])
        mps = [pmm.tile([128, CH], F32, tag="mp", name=f"mp{n}")
               for n in range(NCH)]
        for j in range(3):
            for n in range(NCH):
                nc.tensor.matmul(
                    mps[n][:], lhs[j], wr[j][:, CH * n:CH * (n + 1)],
                    start=(j == 0), stop=(j == 2),
                )
        for n in range(NCH):
            ms = wpool.tile([128, CH], BF16, tag="ms")
            nc.scalar.copy(ms[:], mps[n][:])
            prod = wpool.tile([128, CH], BF16, tag="prod")
            nc.vector.tensor_mul(prod[:], ms[:], c_all[c][:])
            nc.vector.reduce_sum(
                out=mv[:, PG * n:PG * (n + 1)],
                in_=prod[:].rearrange("p (a k) -> p a k", k=R),
                axis=AX.X,
            )
        return mv

    mvt = [branch_mv("t", c) for c in range(NB)]

    logits = cpool.tile([128, NB], F32, tag="logits")
    for c in range(NB):
        mvc = branch_mv("c", c)
        tp = pmisc.tile([MH, 128], F32, tag="pm", name="pmvT")
        nc.tensor.transpose(tp[:], mvt[c][:], ident[:])
        mvtT = lpool.tile([MH, 128], F32, tag="mvtT")
        nc.scalar.copy(mvtT[:], tp[:])
        mg = pmisc.tile([128, MH], F32, tag="pm", name="pmg")
        nc.tensor.matmul(mg[:], mvtT[:], gh[:], start=True, stop=True)
        junk = lpool.tile([128, MH], F32, tag="ttrjunk")
        nc.vector.tensor_mul(junk[:], mg[:], mvc[:])
        nc.vector.reduce_sum(out=logits[:, c:c + 1], in_=junk[:], axis=AX.X)

    # ---- batched loss tail: softplus(l) - l*y over [128, NB] -----------
    ab = lpool.tile([128, NB], F32, tag="ab")
    nc.scalar.activation(ab[:], logits[:], AF.Abs)
    ex = lpool.tile([128, NB], F32, tag="ex")
    nc.scalar.activation(ex[:], ab[:], AF.Exp, scale=-1.0)
    l1p = lpool.tile([128, NB], F32, tag="l1p")
    nc.scalar.activation(l1p[:], ex[:], AF.Ln, bias=1.0)
    rl = lpool.tile([128, NB], F32, tag="rl")
    nc.scalar.activation(rl[:], logits[:], AF.Relu)
    sp = lpool.tile([128, NB], F32, tag="sp")
    nc.vector.tensor_add(sp[:], rl[:], l1p[:])
    ll = lpool.tile([128, NB], F32, tag="ll")
    nc.vector.tensor_mul(ll[:], logits[:], labels[:])
    dvec = lpool.tile([128, NB], F32, tag="dvec")
    nc.vector.tensor_sub(dvec[:], sp[:], ll[:])

    srow = cpool.tile([128, 1], F32, tag="srow")
    nc.vector.reduce_sum(out=srow[:], in_=dvec[:], axis=AX.X)
    fin = pmisc.tile([1, 1], F32, tag="pm", name="pfin")
    nc.tensor.matmul(fin[:], srow[:], ones128[:], start=True, stop=True)
    res = cpool.tile([1, 1], F32, tag="res")
    nc.scalar.copy(res[:], fin[:])
    nc.sync.dma_start(out=io["out"][:, :], in_=res[:])


_PROGRAM = None


def _get_program():
    global _PROGRAM
    if _PROGRAM is not None:
        return _PROGRAM
    nc = bacc.Bacc("TRN2", target_bir_lowering=False, debug=False,
                   num_devices=N_CORES)
    io = {
        "ttab": nc.dram_tensor("ttab", [V, EPAD], BF16, kind="ExternalInput").ap(),
        "ctab": nc.dram_tensor("ctab", [V, EPAD], BF16, kind="ExternalInput").ap(),
        "wr": nc.dram_tensor("wr", [EPAD, NPR], BF16, kind="ExternalInput").ap(),
        "vr": nc.dram_tensor("vr", [H, R], F32, kind="ExternalInput").ap(),
        "h2kb": nc.dram_tensor("h2kb", [H + 1, H], F32, kind="ExternalInput").ap(),
        "h1k": nc.dram_tensor("h1k", [H, 1], F32, kind="ExternalInput").ap(),
        "h1b": nc.dram_tensor("h1b", [H, 1], F32, kind="ExternalInput").ap(),
        "lastkh": nc.dram_tensor("lastkh", [MH, EMB], F32, kind="ExternalInput").ap(),
        "ident": nc.dram_tensor("ident", [128, 128], F32, kind="ExternalInput").ap(),
        "identb": nc.dram_tensor("identb", [128, 128], BF16, kind="ExternalInput").ap(),
        "times": nc.dram_tensor("times", [1, BC], F32, kind="ExternalInput").ap(),
        "labels": nc.dram_tensor("labels", [128, NB], F32, kind="ExternalInput").ap(),
        "idx_t": nc.dram_tensor("idx_t", [128, S_TOT // 16], I16, kind="ExternalInput").ap(),
        "idx_c": nc.dram_tensor("idx_c", [128, S_TOT // 16], I16, kind="ExternalInput").ap(),
        "rel_t": nc.dram_tensor("rel_t", [128, BC // 16], I16, kind="ExternalInput").ap(),
        "rel_c": nc.dram_tensor("rel_c", [128, BC // 16], I16, kind="ExternalInput").ap(),
        "out": nc.dram_tensor("out", [1, 1], F32, kind="ExternalOutput").ap(),
    }
    with tile.TileContext(nc) as tc:
        with ExitStack() as ctx:
            _build_kernel(ctx, tc, io)
    nc.compile()
    _PROGRAM = nc
    return nc


def _pad_table(tab):
    out = np.zeros((V, EPAD), dtype=ml_dtypes.bfloat16)
    out[:, :EMB] = np.asarray(tab).astype(ml_dtypes.bfloat16)
    out[:, EMB] = 1.0
    return out


def _tv_basis(h1_k, h1_b, h2_k, h2_b):
    """Top-R right singular basis of the tv curve (weights-only precompute)."""
    g = np.linspace(0.0, 1.0, 8193, dtype=np.float64).reshape(-1, 1)
    h1 = np.tanh(g @ np.asarray(h1_k, np.float64).reshape(1, H)
                 + np.asarray(h1_b, np.float64).reshape(H))
    tvg = np.tanh(h1 @ np.asarray(h2_k, np.float64)
                  + np.asarray(h2_b, np.float64).reshape(H))
    _, _, vt = np.linalg.svd(tvg, full_matrices=False)
    return np.ascontiguousarray(vt[:R].T)          # [100, R]


def build_in_maps(targets, contexts, times, labels, targetemb, contextemb,
                  h1_k, h1_b, h2_k, h2_b, evoke_k, evoke_b, last_k, last_b):
    ttab = _pad_table(targetemb)
    ctab = _pad_table(contextemb)
    vrb = _tv_basis(h1_k, h1_b, h2_k, h2_b)        # [100, R] float64
    evoke_pad = np.zeros((EPAD, H * H), dtype=np.float64)
    evoke_pad[:EMB, :] = np.asarray(evoke_k, np.float64)
    evoke_pad[EMB, :] = np.asarray(evoke_b, np.float64)
    # Wr[e, (p, k)] = sum_h evoke_pad[e, p*100+h] * Vr[h, k]
    wrm = (evoke_pad.reshape(EPAD * H, H) @ vrb).reshape(EPAD, NPR)
    wrm = wrm.astype(ml_dtypes.bfloat16)
    h2kb = np.vstack([np.asarray(h2_k), np.asarray(h2_b).reshape(1, H)]
                     ).astype(np.float32)
    h1kc = np.asarray(h1_k).reshape(1, H).T.astype(np.float32).copy()
    h1bc = np.asarray(h1_b).reshape(H, 1).astype(np.float32).copy()
    lastkh = np.vstack([np.asarray(last_k), np.asarray(last_b).reshape(1, EMB)]
                       ).astype(np.float32)
    ident = np.eye(128, dtype=np.float32)
    identb = np.eye(128, dtype=ml_dtypes.bfloat16)
    targets = np.asarray(targets)
    contexts = np.asarray(contexts)
    times = np.asarray(times).astype(np.float32)
    labels = np.asarray(labels).astype(np.float32)

    in_maps = []
    for k in range(N_CORES):
        sl = slice(k * BC, (k + 1) * BC)
        idx_t, rel_t = _prep_indices(targets[sl])
        idx_c, rel_c = _prep_indices(contexts[sl])
        in_maps.append({
            "ttab": ttab, "ctab": ctab, "wr": wrm,
            "vr": vrb.astype(np.float32), "h2kb": h2kb,
            "h1k": h1kc, "h1b": h1bc, "lastkh": lastkh, "ident": ident,
            "identb": identb,
            "times": times[sl].reshape(1, BC),
            "labels": labels[sl].reshape(NB, 128).T.copy(),
            "idx_t": idx_t, "idx_c": idx_c, "rel_t": rel_t, "rel_c": rel_c,
        })
    return in_maps


def kernel(**inputs) -> np.ndarray:
    nc = _get_program()
    in_maps = build_in_maps(**inputs)
    r = run_bass_kernel_spmd(nc, in_maps, list(range(N_CORES)))
    total = np.float64(0.0)
    for m in r.results:
        total += np.float64(m["out"][0, 0])
    return np.float32(total / B)
